# revision 38
# baseline (speedup 1.0000x reference)
"""MLA-style causal self-attention on 8 Trainium2 NeuronCores.

Sharding: tensor-parallel over heads (2 heads/core) for W_qdec/W_out, and
the latent encode (phase A) is data-parallel over the sequence: each core
computes latents^T for its own 256-column T shard (1/8 of the FLOPs) and
two HBM AllGathers (c_q rows first, then kv+rope rows) rebuild the full
latents on every core. The collectives run on TOPSP/SDMA silicon so they
overlap with PE work. Each core returns a partial out^T [E, T]; the host
sums the 8 partials and transposes.

The whole datapath runs in bf16 (fp32 PSUM accumulation everywhere), which
halves DMA + AllGather bytes, halves LDWEIGHTS time (FWL applies to 16-bit
weights), and draws less PE power (the fp32r version tripped the gpio
power throttle to ~70% effective clock).

Phase order is arranged so the PE never waits on a collective longer than
necessary:
  A:    latents^T shard = W_qkv^T @ x^T[:, shard]; the 8 c_q row-groups
        are computed first and AllGather #1 (c_q) is issued mid-phase;
        the 5 kv+rope groups follow with AllGather #2 (kv) at the end.
  B:    all q^T chunks for both heads (depends only on AG #1): per
        512-query chunk, q^T = W_qdec_h^T @ c_q^T with SCALE folded in,
        rope subchunk first (64x64 permutation matmul + DVE chain).
  preC: rope k_r^T, v = (c_kv^T)^T via PE transposes (AG #2 done by now).
  C:    flash-style causal attention per (head, 512-query chunk):
        st[k,q] accumulated over 5 key matmuls, exp on ScalarE,
        edge-masked, denominator accumulated on VectorE into fp32 with a
        single closing ones-matmul, y^T[d,q] in 4 PSUM banks, normalized,
        staged to per-chunk DRAM tiles.
  D:    out^T = W_out_c^T @ y^T -> HBM (kc-major for stationary reuse).
"""

import math
from contextlib import ExitStack

import ml_dtypes
import numpy as np

import concourse.bass as bass
import concourse.tile as tile
from concourse import bacc, mybir
from concourse.bass_utils import run_bass_kernel_spmd
from concourse.masks import make_identity

F32 = mybir.dt.float32
F32R = mybir.dt.float32r
BF = mybir.dt.bfloat16
AF = mybir.ActivationFunctionType
NPBF = ml_dtypes.bfloat16

# Problem constants (hardcoded per harness contract)
T_FULL = 2048
E = 2048          # n_embd
KV = 512          # kv low rank == head size
QL = 1024         # q low rank
RH = 64           # rope head size
QKH = KV + RH     # 576
NH = 16
NCORES = 8
HPC = NH // NCORES  # heads per core
SCALE = 1.0 / math.sqrt(float(KV))

P = 128
RG = [list(range(NCORES))]


def _make_rot64(nc, pool):
    """RT [64, 64] permutation with RT[x, y] = 1 iff x == (y+32) % 64, so
    matmul(out, lhsT=RT, rhs=src) gives out[d] = src[(d+32) % 64]."""
    rt0 = pool.tile([RH, RH], F32, tag="rt0")
    nc.gpsimd.memset(rt0[:], 0.0)
    # fill 1 where x - y - 32 == 0
    nc.gpsimd.affine_select(
        out=rt0[:], in_=rt0[:], compare_op=mybir.AluOpType.not_equal,
        fill=1.0, base=-32, channel_multiplier=1, pattern=[[-1, RH]],
    )
    # fill 1 where x - y + 32 == 0
    nc.gpsimd.affine_select(
        out=rt0[:], in_=rt0[:], compare_op=mybir.AluOpType.not_equal,
        fill=1.0, base=32, channel_multiplier=1, pattern=[[-1, RH]],
    )
    rt = pool.tile([RH, RH], BF, tag="rt")
    nc.vector.tensor_copy(rt[:], rt0[:])
    return rt


def build_kernel(T=T_FULL):
    """Build the single-core program (SPMD across 8 cores via per-core data)."""
    assert T % 512 == 0
    NT512 = T // 512
    NKT = T // P          # key tiles of 128
    EK = E // P           # 16 contraction chunks for phase A
    TS = T // NCORES      # per-core T shard for phase A

    nc = bacc.Bacc("TRN2", target_bir_lowering=False, debug=False,
                   num_devices=NCORES)

    xTs = nc.dram_tensor("xTs", [E, TS], BF, kind="ExternalInput").ap()
    wqkv = nc.dram_tensor("wqkv", [E, QKH + QL], BF, kind="ExternalInput").ap()
    wqdec = nc.dram_tensor("wqdec", [QL, HPC * QKH], BF, kind="ExternalInput").ap()
    wout = nc.dram_tensor("wout", [HPC * KV, E], BF, kind="ExternalInput").ap()
    cosd = nc.dram_tensor("cosT", [RH, T], BF, kind="ExternalInput").ap()
    sind = nc.dram_tensor("sinT", [RH, T], BF, kind="ExternalInput").ap()
    outT = nc.dram_tensor("outT", [E, T], BF, kind="ExternalOutput").ap()

    xts_r = xTs.rearrange("(ko p) t -> p ko t", p=P)
    wq_r = wqkv.rearrange("(ko p) m -> p ko m", p=P)

    # phase A row groups: 8x128 c_q first (their AllGather gates phase B),
    # then 4x128 c_kv + 1x64 k_r
    groups = [(QKH + i * P, P) for i in range(QL // P)] + [
        (i * P, P) for i in range(KV // P)
    ] + [(KV, RH)]
    NCQ = QL // P

    with tile.TileContext(nc) as tc, ExitStack() as ctx:
        dram = ctx.enter_context(tc.tile_pool(name="dram", bufs=1, space="DRAM"))
        cst = ctx.enter_context(tc.tile_pool(name="cst", bufs=1))
        kvp = ctx.enter_context(tc.tile_pool(name="kvp", bufs=1))
        pp = ctx.enter_context(tc.tile_pool(name="pp", bufs=3, space="PSUM"))
        ppy = ctx.enter_context(tc.tile_pool(name="ppy", bufs=1, space="PSUM"))

        # latents bounce (this core's shard) + AllGather outputs (full T).
        # two collectives only -- small AllGathers pay a big per-call floor
        bkv = dram.tile([QKH, TS], BF, tag="bkv")
        bcq = dram.tile([QL, TS], BF, tag="bcq")
        agkv = dram.tile([NCORES, QKH, TS], BF, tag="agkv",
                         addr_space="Shared")
        agcq = dram.tile([NCORES, QL, TS], BF, tag="agcq",
                         addr_space="Shared")

        # ---- global constants / kv residents ----
        ident0 = cst.tile([P, P], F32, tag="ident0")
        make_identity(nc, ident0[:])
        ident = cst.tile([P, P], BF, tag="ident")
        nc.vector.tensor_copy(ident[:], ident0[:])
        ones0 = cst.tile([P, 1], F32, tag="ones0")
        nc.gpsimd.memset(ones0[:], 1.0)
        ones_col = cst.tile([P, 1], F32R, tag="ones")
        nc.vector.tensor_copy(ones_col[:], ones0[:])

        ckvT = kvp.tile([P, KV // P, T], BF, tag="ckvT")  # c_kv^T
        krT = kvp.tile([RH, T], BF, tag="krT")            # k_r^T (pre-rope)
        krT2 = kvp.tile([RH, T], BF, tag="krT2")          # k_r^T (roped)
        # y^T lives in SBUF end-to-end (C writes it, D contracts it)
        yT_sb = [kvp.tile([P, T], BF, tag=f"yT{k}", name=f"yT{k}")
                 for k in range(HPC * KV // P)]

        # ====== Phase A: latents^T shard = W_qkv^T @ x^T[:, shard] ========
        with ExitStack() as actx:
            aw = actx.enter_context(tc.tile_pool(name="aw", bufs=1))
            asr = actx.enter_context(tc.tile_pool(name="asr", bufs=1))
            astp = actx.enter_context(tc.tile_pool(name="astp", bufs=3))

            # x shard first on the sync queue so it isn't stuck behind the
            # W_qkv chunks
            xts = asr.tile([P, EK, TS], BF, tag="xts", name="xts")
            nc.sync.dma_start(xts[:], xts_r[:])

            # W on scalar+sync only: the gpsimd queue is reserved for the
            # bounce DMAs so the AllGather triggers aren't stuck behind
            # megabytes of queued weight traffic. Quarter-granularity
            # chunks let the first matmuls chase the arriving weights.
            EH = EK // 2
            EQ = EK // 4
            wqt = []
            for gi, (c0, M) in enumerate(groups):
                wa = aw.tile([P, EH, M], BF, tag=f"wqa{gi}", name=f"wqa{gi}")
                wb = aw.tile([P, EH, M], BF, tag=f"wqb{gi}", name=f"wqb{gi}")
                nc.scalar.dma_start(wa[:, 0:EQ, :], wq_r[:, 0:EQ, c0 : c0 + M])
                nc.scalar.dma_start(wa[:, EQ:EH, :],
                                    wq_r[:, EQ:EH, c0 : c0 + M])
                nc.sync.dma_start(wb[:, 0:EQ, :],
                                  wq_r[:, EH : EH + EQ, c0 : c0 + M])
                nc.sync.dma_start(wb[:, EQ:EH, :],
                                  wq_r[:, EH + EQ : EK, c0 : c0 + M])
                wqt.append((wa, wb))

            for gi, (c0, M) in enumerate(groups):
                ps = pp.tile([P, TS], F32, tag="mm", name="psA")
                for kc in range(EK):
                    ws = (wqt[gi][0][:, kc, :] if kc < EH
                          else wqt[gi][1][:, kc - EH, :])
                    nc.tensor.matmul(
                        ps[:M], ws, xts[:, kc, :],
                        start=(kc == 0), stop=(kc == EK - 1),
                    )
                st = astp.tile([P, TS], BF, tag="ast", name="ast")
                if gi % 2 == 0:
                    nc.vector.tensor_copy(st[:M], ps[:M])
                else:
                    nc.scalar.copy(st[:M], ps[:M])
                if c0 < QKH:
                    nc.gpsimd.dma_start(bkv[c0 : c0 + M, :], st[:M])
                else:
                    nc.gpsimd.dma_start(
                        bcq[c0 - QKH : c0 - QKH + M, :], st[:M])
                if gi == NCQ - 1:  # c_q rows staged -> gather them first
                    nc.gpsimd.collective_compute(
                        "AllGather", mybir.AluOpType.bypass,
                        replica_groups=RG, ins=[bcq.opt()], outs=[agcq.opt()],
                    )
            nc.gpsimd.collective_compute(
                "AllGather", mybir.AluOpType.bypass,
                replica_groups=RG, ins=[bkv.opt()], outs=[agkv.opt()],
            )

        # ---- load kv residents (full T) from the kv AllGather output.
        # gpsimd queue: these wait on the kv AllGather no matter what, and
        # keeping them off sync/scalar stops them from head-of-line
        # blocking the c_q loads phase B needs much earlier.
        for dc in range(KV // P):
            nc.gpsimd.dma_start(
                ckvT[:, dc, :].rearrange("p (r t) -> p r t", r=NCORES),
                agkv[:, dc * P : (dc + 1) * P, :].rearrange("r p t -> p r t"),
            )
        nc.gpsimd.dma_start(
            krT[:].rearrange("p (r t) -> p r t", r=NCORES),
            agkv[:, KV : KV + RH, :].rearrange("r p t -> p r t"),
        )

        # ============ Phases B + preC + C =================================
        with ExitStack() as bctx:
            bcp = bctx.enter_context(tc.tile_pool(name="bcp", bufs=1))
            bcs = bctx.enter_context(tc.tile_pool(name="bcs", bufs=2))
            qq = bctx.enter_context(tc.tile_pool(name="qq", bufs=1))

            # constants: rope tables, rotation matrix, edge masks
            rt = _make_rot64(nc, bcp)
            masks = []
            for r in range(4):
                m0 = bcp.tile([P, 512], BF, tag=f"mask{r}", name=f"mask{r}")
                nc.gpsimd.memset(m0[:], 1.0)
                nc.gpsimd.affine_select(
                    out=m0[:], in_=m0[:], compare_op=mybir.AluOpType.is_ge,
                    fill=0.0, base=-P * r, channel_multiplier=-1,
                    pattern=[[1, 512]],
                )
                masks.append(m0)
            cosT = bcp.tile([RH, T], BF, tag="cosT")
            nc.scalar.dma_start(cosT[:], cosd[:])
            ssinT = bcp.tile([RH, T], BF, tag="ssinT")
            nc.scalar.dma_start(ssinT[:], sind[:])
            nc.vector.tensor_scalar_mul(
                ssinT[0 : RH // 2, :], ssinT[0 : RH // 2, :], -1.0
            )

            # stacked-rope constants: both heads' 64-row rope decode shares
            # one 128-row matmul chain. block-diag(rt, rt) rotation (second
            # block copied via DMA -- cross-partition), doubled cos/sin.
            RH2 = 2 * RH
            rt2_0 = bcp.tile([RH2, RH2], F32, tag="rt2_0")
            nc.gpsimd.memset(rt2_0[:], 0.0)
            nc.gpsimd.affine_select(
                out=rt2_0[0:RH, 0:RH], in_=rt2_0[0:RH, 0:RH],
                compare_op=mybir.AluOpType.not_equal,
                fill=1.0, base=-32, channel_multiplier=1, pattern=[[-1, RH]],
            )
            nc.gpsimd.affine_select(
                out=rt2_0[0:RH, 0:RH], in_=rt2_0[0:RH, 0:RH],
                compare_op=mybir.AluOpType.not_equal,
                fill=1.0, base=32, channel_multiplier=1, pattern=[[-1, RH]],
            )
            nc.gpsimd.dma_start(rt2_0[RH:RH2, RH:RH2], rt2_0[0:RH, 0:RH])
            rt2 = bcp.tile([RH2, RH2], BF, tag="rt2")
            nc.vector.tensor_copy(rt2[:], rt2_0[:])
            cos2 = bcp.tile([RH2, T], BF, tag="cos2")
            nc.scalar.dma_start(cos2[0:RH, :], cosd[:])
            nc.scalar.dma_start(cos2[RH:RH2, :], cosd[:])
            ssin2 = bcp.tile([RH2, T], BF, tag="ssin2")
            nc.scalar.dma_start(ssin2[0:RH, :], sind[:])
            nc.scalar.dma_start(ssin2[RH:RH2, :], sind[:])
            nc.vector.tensor_scalar_mul(
                ssin2[0 : RH // 2, :], ssin2[0 : RH // 2, :], -1.0
            )
            nc.vector.tensor_scalar_mul(
                ssin2[RH : RH + RH // 2, :], ssin2[RH : RH + RH // 2, :], -1.0
            )
            # combined rope columns of both heads' W_qdec
            wqr = bcp.tile([P, QL // P, RH2], BF, tag="wqr")
            for h in range(HPC):
                nc.scalar.dma_start(
                    wqr[:, :, h * RH : (h + 1) * RH],
                    wqdec.rearrange("(ko p) m -> p ko m", p=P)[
                        :, :, h * QKH + KV : h * QKH + QKH
                    ],
                )

            # both heads' decode weights, early, off the gpsimd queue (the
            # collectives hold that queue's ordering)
            wqds = []
            for h in range(HPC):
                wqd = bcp.tile([P, QL // P, QKH], BF, tag=f"wqd{h}",
                               name=f"wqd{h}")
                nc.scalar.dma_start(
                    wqd[:],
                    wqdec.rearrange("(ko p) m -> p ko m", p=P)[
                        :, :, h * QKH : (h + 1) * QKH
                    ],
                )
                wqds.append(wqd)

            # ---- B: all q^T chunks (both heads) -- only needs the c_q AG.
            # stacked rope chain first (both heads in one 128-row group) so
            # its DVE work hides under the nope groups
            qgroups = [(i * P, P) for i in range(KV // P)]
            qT = [[None] * NT512 for _ in range(HPC)]
            for i4 in range(NT512):
                qsl = slice(i4 * 512, (i4 + 1) * 512)
                cq = bcs.tile([P, QL // P, 512], BF, tag="cq", bufs=3,
                              name="cq")
                for rb in range(2):
                    nc.sync.dma_start(
                        cq[:, :, rb * 256 : (rb + 1) * 256],
                        agcq[2 * i4 + rb].rearrange("(ko p) t -> p ko t",
                                                    p=P),
                    )
                # rope decode + rotate for BOTH heads, one chain
                psr = pp.tile([RH2, 512], F32, tag="mm", name="psR")
                for kc in range(QL // P):
                    nc.tensor.matmul(
                        psr[:], wqr[:, kc, :], cq[:, kc, :],
                        start=(kc == 0), stop=(kc == QL // P - 1),
                    )
                qr2 = bcs.tile([RH2, 512], BF, tag="qr2", name="qr2")
                nc.vector.tensor_scalar_mul(qr2[:], psr[:], SCALE)
                pr = pp.tile([RH2, 512], F32, tag="mm", name="prq")
                nc.tensor.matmul(pr[:], rt2[:], qr2[:], start=True, stop=True)
                qrS = qq.tile([RH2, 512], BF, tag=f"qrS{i4}",
                              name=f"qrS{i4}")
                nc.vector.tensor_mul(qrS[:], qr2[:], cos2[:, qsl])
                rot = bcs.tile([RH2, 512], BF, tag="rot", name="rotq")
                nc.vector.tensor_mul(rot[:], pr[:], ssin2[:, qsl])
                nc.vector.tensor_add(qrS[:], qrS[:], rot[:])
                # h0 reads the top half in place; h1's half moves to
                # partitions 0..63 via DMA (DVE can't cross partitions)
                qrT1 = qq.tile([RH, 512], BF, tag=f"qrT1_{i4}",
                               name=f"qrT1_{i4}")
                nc.scalar.dma_start(qrT1[:], qrS[RH:RH2, :])
                for h in range(HPC):
                    qTc = [qq.tile([P, 512], BF, tag=f"qTc{h}_{i4}_{i}",
                                   name=f"qTc{h}_{i4}_{i}")
                           for i in range(KV // P)]
                    for (m0, M) in qgroups:
                        ps = pp.tile([P, 512], F32, tag="mm", name="psB")
                        for kc in range(QL // P):
                            nc.tensor.matmul(
                                ps[:M], wqds[h][:, kc, m0 : m0 + M],
                                cq[:, kc, :],
                                start=(kc == 0), stop=(kc == QL // P - 1),
                            )
                        nc.vector.tensor_scalar_mul(
                            qTc[m0 // P][:], ps[:], SCALE
                        )
                    qT[h][i4] = (qTc, qrS[0:RH, :] if h == 0 else qrT1[:])

            # ---- preC: rope k_r, v transposes (AG #2 done long ago) ------
            for tcc in range(NT512):
                tsl = slice(tcc * 512, (tcc + 1) * 512)
                pr = pp.tile([RH, 512], F32, tag="mm", name="prk")
                nc.tensor.matmul(pr[:], rt[:], krT[:, tsl],
                                 start=True, stop=True)
                nc.vector.tensor_mul(krT2[:, tsl], krT[:, tsl], cosT[:, tsl])
                rot = bcs.tile([RH, 512], BF, tag="rot", name="rotk")
                nc.vector.tensor_mul(rot[:], pr[:], ssinT[:, tsl])
                nc.vector.tensor_add(krT2[:, tsl], krT2[:, tsl], rot[:])

            v = bcp.tile([P, NKT, KV], BF, tag="v")
            with tc.tile_pool(name="ptr", bufs=1, space="PSUM") as ptr:
                for dc in range(KV // P):
                    for tt in range(NKT):
                        pt = ptr.tile([P, P], BF, tag="tr", name="pt")
                        nc.tensor.transpose(
                            pt[:], ckvT[:, dc, tt * P : (tt + 1) * P], ident[:]
                        )
                        nc.vector.tensor_copy(
                            v[:, tt, dc * P : (dc + 1) * P], pt[:]
                        )
            pden = bctx.enter_context(
                tc.tile_pool(name="pden", bufs=1, space="PSUM")
            )

            # zero the rotating se buffers once: trimmed tiles rely on the
            # full-width mask multiply to zero their [0, cut) region, and
            # 0 x uninitialized-NaN would poison it on first use
            for _ in range(3):
                se0 = bcs.tile([P, 512], BF, tag="se", bufs=3, name="se")
                nc.gpsimd.memset(se0[:], 0.0)

            # ---- C: causal attention per (head, 512-query chunk) ---------
            for h in range(HPC):
                for i4 in range(NT512):
                    qsl = slice(i4 * 512, (i4 + 1) * 512)
                    qTc, qrT = qT[h][i4]
                    nj = 4 * i4 + 4
                    sden = bcs.tile([P, 512], F32R, tag="sden", name="sden")
                    psy = [ppy.tile([P, 512], F32, tag=f"y{dc}",
                                    name=f"psy{dc}")
                           for dc in range(KV // P)]
                    # key tiles in DESCENDING order: the diagonal tiles run
                    # first with trimmed query ranges (their masked-out
                    # columns are provably zero), and the first (full-width,
                    # start=True) y matmul zero-initializes the whole PSUM
                    # so later trimmed accumulations are correct.
                    for jj in range(nj):
                        j = nj - 1 - jj
                        ksl = slice(j * P, (j + 1) * P)
                        r = j - 4 * i4
                        cut = r * P if r >= 1 else 0
                        csl = slice(cut, 512)
                        ps = pp.tile([P, 512], F32, tag="mm", name="psS")
                        for dc in range(KV // P):
                            nc.tensor.matmul(
                                ps[:, csl], ckvT[:, dc, ksl], qTc[dc][:, csl],
                                start=(dc == 0), stop=False,
                            )
                        nc.tensor.matmul(
                            ps[:, csl], krT2[:, ksl], qrT[:, csl],
                            start=False, stop=True,
                        )
                        se = bcs.tile([P, 512], BF, tag="se", bufs=3,
                                      name="se")
                        nc.scalar.activation(se[:, csl], ps[:, csl], AF.Exp)
                        if r >= 0:
                            # full-width: also zeroes the stale [0, cut)
                            # region left from this buffer's previous use
                            nc.vector.tensor_mul(se[:], se[:], masks[r][:])
                        # denominator accumulates on VectorE in fp32 (PE
                        # stays on score/y matmuls)
                        if jj == 0:
                            nc.vector.tensor_copy(sden[:], se[:])
                        else:
                            nc.vector.tensor_add(sden[:], sden[:], se[:])
                        ysl = slice(0, 512) if jj == 0 else csl
                        for dc in range(KV // P):
                            nc.tensor.matmul(
                                psy[dc][:, ysl],
                                v[:, j, dc * P : (dc + 1) * P],
                                se[:, ysl],
                                start=(jj == 0), stop=(j == 0),
                            )
                    psden = pden.tile([1, 512], F32, tag="den", name="psden")
                    nc.tensor.matmul(psden[:], ones_col[:], sden[:],
                                     start=True, stop=True)
                    deninv = bcs.tile([1, 512], F32, tag="deninv",
                                      name="deninv")
                    nc.vector.reciprocal_approx_fast(out=deninv[:],
                                                     in_=psden[:])
                    denb = bcs.tile([P, 512], F32, tag="denb", name="denb")
                    nc.gpsimd.partition_broadcast(denb[:], deninv[:])
                    for dc in range(KV // P):
                        nc.vector.tensor_mul(
                            yT_sb[h * (KV // P) + dc][:, qsl],
                            psy[dc][:], denb[:],
                        )

        # ================= Phase D: out^T = W_out_c^T @ y^T ===============
        with ExitStack() as dctx:
            dwp = dctx.enter_context(tc.tile_pool(name="dwp", bufs=2))
            dst = dctx.enter_context(tc.tile_pool(name="dst", bufs=3))

            DK = HPC * KV // P  # 8 contraction chunks (y^T already in SBUF)
            wo_r = wout.rearrange("(ko p) e -> p ko e", p=P)
            for mc in range(E // P):
                wo = dwp.tile([P, DK, P], BF, tag="wo", name="wo")
                nc.sync.dma_start(wo[:], wo_r[:, :, mc * P : (mc + 1) * P])
                psD = [ppy.tile([P, 512], F32, tag=f"y{tcc}", name=f"psD{tcc}")
                       for tcc in range(NT512)]
                # kc-major: the stationary operand is reused across the 4
                # consecutive matmuls, letting LDWEIGHTS pull-ahead hide
                for kc in range(DK):
                    for tcc in range(NT512):
                        nc.tensor.matmul(
                            psD[tcc][:], wo[:, kc, :],
                            yT_sb[kc][:, tcc * 512 : (tcc + 1) * 512],
                            start=(kc == 0), stop=(kc == DK - 1),
                        )
                    if kc == DK - 1:
                        # copies chase the closing matmuls, split DVE/ACT
                        for tcc in range(NT512):
                            ost = dst.tile([P, 512], BF, tag=f"ost{tcc % 2}",
                                           name="ost")
                            if tcc % 2 == 0:
                                nc.vector.tensor_copy(ost[:], psD[tcc][:])
                            else:
                                nc.scalar.copy(ost[:], psD[tcc][:])
                            # spread the 8MB output stream over 3 queues so
                            # the final drain doesn't serialize on one
                            eng = (nc.scalar, nc.sync, nc.gpsimd)[
                                (mc * NT512 + tcc) % 3]
                            eng.dma_start(
                                outT[mc * P : (mc + 1) * P,
                                     tcc * 512 : (tcc + 1) * 512], ost[:]
                            )

    nc.compile()
    return nc


_NC_CACHE = {}


def _get_nc(T=T_FULL):
    if T not in _NC_CACHE:
        _NC_CACHE[T] = build_kernel(T)
    return _NC_CACHE[T]


def make_in_maps(x, cos, sin, W_qkv, W_qdec, W_out):
    """Host-side sharding/layout: transpose activations & tables, cast to
    bf16, slice the head-parallel weights and the per-core x shard."""
    xT = np.ascontiguousarray(np.asarray(x)[0].T).astype(NPBF)
    T = xT.shape[1]
    TS = T // NCORES
    cosT = np.ascontiguousarray(np.asarray(cos).T).astype(NPBF)
    sinT = np.ascontiguousarray(np.asarray(sin).T).astype(NPBF)
    W_qkv = np.ascontiguousarray(np.asarray(W_qkv)).astype(NPBF)
    W_qdec = np.asarray(W_qdec).astype(NPBF)
    W_out = np.asarray(W_out).astype(NPBF)
    in_maps = []
    for c in range(NCORES):
        in_maps.append({
            "xTs": np.ascontiguousarray(xT[:, c * TS : (c + 1) * TS]),
            "wqkv": W_qkv,
            "wqdec": np.ascontiguousarray(
                W_qdec[:, c * HPC * QKH : (c + 1) * HPC * QKH]
            ),
            "wout": np.ascontiguousarray(
                W_out[c * HPC * KV : (c + 1) * HPC * KV]
            ),
            "cosT": cosT,
            "sinT": sinT,
        })
    return in_maps


def kernel(x, cos, sin, W_qkv, W_qdec, W_out, _trace=False, _tmpdir=None):
    T = np.asarray(x).shape[1]
    nc = _get_nc(T)
    in_maps = make_in_maps(x, cos, sin, W_qkv, W_qdec, W_out)
    res = run_bass_kernel_spmd(
        nc, in_maps, core_ids=list(range(NCORES)),
        trace=_trace, tmpdir=_tmpdir,
    )
    out = np.zeros((E, T), np.float32)
    for r in res.results:
        out += np.asarray(r["outT"]).astype(np.float32)
    kernel.last_results = res
    return np.ascontiguousarray(out.T)[None].astype(np.float32)
